# revision 47
# baseline (speedup 1.0000x reference)
"""Bahdanau additive attention (vectorized) on TRN2 — Bass/Tile kernel.

Problem: nn_AttentionLayer_11055245820581
  e[b,y,x] = softmax_x( sum_e V[e] * tanh(Ws[b,x,e] + Uh[b,y,e]) )
  c[b,y,:] = sum_x e[b,y,x] * enc[b,x,:]
with Ws = enc @ W_a, Uh = dec @ U_a.

Sharding: data-parallel over batch B=8 across the 8 NeuronCores (one
batch element per core). Each core computes its batch's full attention.

Per-core dataflow (the tanh cube Ty*Tx*E = 16.7M elements dominates;
ACT's 1 elem/lane/cycle tanh is the ~110us floor, everything else is
arranged to stay below it; measured ~155us/kernel on HW):
  - broadcast-add WsT[e,x] + UhT[e,y] into fp16 slabs, split per
    (y-block, e-chunk) between DVE (tensor_scalar_add with per-partition
    fp32 scalar, ~283ns per 256-elem op) and the Tensor engine (identity
    matmuls of a step-0-broadcast W plus an inner-broadcast U summed in
    double-buffered PSUM pieces that ACT tanh-reads directly).
  - ACT: one big fp16 Tanh per (y-block, chunk) DVE slab + one per PSUM
    piece; instruction count kept low (352-cycle fixed cost per op).
  - PE: projection with the tanh slab as fp16 stationary [128e, 128x]
    and V fp16 moving (N=1): e'^T lands as [x(partition), y] columns in
    per-y-half PSUM tiles (no PSUM evacuation, M=128 amortizes LDW).
  - softmax per y-half in the transposed layout, overlapped with the
    other half's main loop: ACT Exp -> expT in SBUF; sum over x via
    matmul with a ones vector -> denom[y]; DVE reciprocal; context
    matmul uses unnormalized expT and scales c rows by 1/denom;
    attention weights are PE-transposed back to [y, x] and scaled.
"""

import numpy as np
from contextlib import ExitStack

import concourse.bass as bass
import concourse.bacc as bacc
import concourse.tile as tile
from concourse import mybir
from concourse.bass_utils import run_bass_kernel_spmd

B, Tx, Ty, E, D = 8, 256, 256, 256, 256
P = 128
NCORES = 8
F32 = mybir.dt.float32
F16 = mybir.dt.float16
TANH = mybir.ActivationFunctionType.Tanh
EXP = mybir.ActivationFunctionType.Exp

EC = E // P      # 2 e-chunks
XC = Tx // P     # 2 x-chunks
YC = Ty // P     # 2 y-halves
DC = D // P      # 2 d-chunks

_NC = None
LAST_RESULTS = None


def _bcast_add_ap(t, n_rep, n_inner):
    """AP reading a [P, n_inner] tile as [P, n_rep, n_inner] (repeat dim 1)."""
    return bass.AP(tensor=t.tensor, offset=t.offset,
                   ap=[t.ap[0], [0, n_rep], t.ap[1]])


def _bcast_inner_ap(t, col0, n_rep, n_inner):
    """AP reading tile columns [col0:col0+n_rep] as [P, n_rep, n_inner]
    (each column repeated n_inner times along the innermost dim)."""
    step = t.ap[1][0]
    return bass.AP(tensor=t.tensor, offset=t.offset + col0 * step,
                   ap=[t.ap[0], [step, n_rep], [0, n_inner]])


def _build_body(tc, ctx, enc_d, dec_d, W_d, U_d, V_d, c_d, e_d):
    nc = tc.nc
    from concourse.masks import make_identity

    consts = ctx.enter_context(tc.tile_pool(name="consts", bufs=1))
    add_pool = ctx.enter_context(tc.tile_pool(name="adds", bufs=4))
    tanh_pool = ctx.enter_context(tc.tile_pool(name="tanhs", bufs=4))
    out_pool = ctx.enter_context(tc.tile_pool(name="outs", bufs=2))
    e_psum = ctx.enter_context(tc.tile_pool(name="pe", bufs=1, space="PSUM"))
    piece_psum = ctx.enter_context(tc.tile_pool(name="ppiece", bufs=2, space="PSUM"))
    misc_psum = piece_psum  # setup/final tiles rotate through the piece slots

    # ---- load inputs ----
    enc_sb = consts.tile([P, XC, E], F32)    # [x_in_chunk, (xc), e]
    dec_sb = consts.tile([P, YC, D], F32)
    W_sb = consts.tile([P, EC, E], F32)      # rows e_in
    U_sb = consts.tile([P, DC, E], F32)      # rows d
    V_sb = consts.tile([P, EC], F32)
    for i in range(XC):
        nc.sync.dma_start(out=enc_sb[:, i, :], in_=enc_d[i * P:(i + 1) * P, :])
    for i in range(YC):
        nc.sync.dma_start(out=dec_sb[:, i, :], in_=dec_d[i * P:(i + 1) * P, :])
    for i in range(EC):
        nc.sync.dma_start(out=W_sb[:, i, :], in_=W_d[i * P:(i + 1) * P, :])
    for i in range(DC):
        nc.sync.dma_start(out=U_sb[:, i, :], in_=U_d[i * P:(i + 1) * P, :])
    for i in range(EC):
        nc.sync.dma_start(out=V_sb[:, i:i + 1], in_=V_d[i * P:(i + 1) * P, :])

    ident = consts.tile([P, P], F32)
    make_identity(nc, ident)
    ident16 = consts.tile([P, P], F16)
    nc.vector.tensor_copy(ident16[:], ident[:])
    ones_sb = consts.tile([P, 1], F32)
    nc.vector.memset(ones_sb[:], 1.0)
    V16_sb = consts.tile([P, EC], F16)
    nc.vector.tensor_copy(V16_sb[:], V_sb[:])
    # Trigger the ACT tanh table load during the otherwise-idle prologue.
    warm_sb = consts.tile([P, 1], F32)
    nc.scalar.activation(out=warm_sb[:], in_=ones_sb[:], func=TANH)

    # ---- transpose enc, dec (PE transpose via identity) ----
    encT_sb = consts.tile([P, EC, Tx], F32)  # [e, (ec), x]
    decT_sb = consts.tile([P, DC, Ty], F32)  # [d, (dc), y]
    for src, srcC, dstT, dstC in ((enc_sb, XC, encT_sb, EC),
                                  (dec_sb, YC, decT_sb, DC)):
        for i in range(srcC):          # source partition chunk (x or y)
            for j in range(dstC):      # source free chunk (e or d)
                pt = misc_psum.tile([P, Tx], F32, tag="piece", name="pt")
                nc.tensor.transpose(
                    out=pt[:, :P], in_=src[:, i, j * P:(j + 1) * P],
                    identity=ident[:])
                nc.scalar.copy(dstT[:, j, i * P:(i + 1) * P], pt[:, :P])

    # ---- WsT[e_out, x] = sum_ei W[ei, e_out] * encT[ei, x] ----
    # fp16 WsT/UhT feed the DVE/PE adds; fp32 UhT feeds the DVE
    # per-partition scalar reads (TensorScalar requires fp32 scalars).
    WsT16_sb = consts.tile([P, EC, Tx], F16)
    UhT16_sb = consts.tile([P, EC, Ty], F16)
    UhT_sb = consts.tile([P, EC, Ty], F32)
    for co in range(EC):
        pw = misc_psum.tile([P, Tx], F32, tag="piece", name="pw")
        for ci in range(EC):
            nc.tensor.matmul(
                out=pw[:], lhsT=W_sb[:, ci, co * P:(co + 1) * P],
                rhs=encT_sb[:, ci, :], start=(ci == 0), stop=(ci == EC - 1))
        nc.scalar.copy(WsT16_sb[:, co, :], pw[:])
    for co in range(EC):
        pu = misc_psum.tile([P, Ty], F32, tag="piece", name="pu")
        for ci in range(DC):
            nc.tensor.matmul(
                out=pu[:], lhsT=U_sb[:, ci, co * P:(co + 1) * P],
                rhs=decT_sb[:, ci, :], start=(ci == 0), stop=(ci == DC - 1))
        nc.scalar.copy(UhT_sb[:, co, :], pu[:])
        nc.scalar.copy(UhT16_sb[:, co, :], pu[:])

    # ---- main loop: tanh cube + V projection into e'^T ----
    # e'^T[x, (xc, y)] accumulates into one [128, XC*128] PSUM tile per
    # y-half (1 bank each) so each half's softmax can start while the
    # other half is still being produced.
    eT_yh = [e_psum.tile([P, XC, P], F32, tag=f"e{h}", name=f"eT_yh{h}")
             for h in range(YC)]
    for h in range(YC):
        nc.vector.memset(eT_yh[h][:], 0.0)

    # ---- per-y-half softmax + context + attention-weight output ----
    expT_sb = consts.tile([P, XC, Ty], F32)  # [x, (xc), y]
    recip_sb = consts.tile([P, YC], F32)
    alpha_sb = consts.tile([P, YC, Tx], F32)

    def _final_half(yh):
        for xc in range(XC):
            nc.scalar.activation(out=expT_sb[:, xc, yh * P:(yh + 1) * P],
                                 in_=eT_yh[yh][:, xc, :], func=EXP)
        # Reuse the just-released eT bank of this half for the final
        # tiles (borrowing piece slots here starves ACT of pieces).
        den = e_psum.tile([P, 1], F32, tag=f"e{yh}", name=f"den{yh}")
        for xc in range(XC):
            nc.tensor.matmul(
                out=den[:],
                lhsT=expT_sb[:, xc, yh * P:(yh + 1) * P],
                rhs=ones_sb[:],
                start=(xc == 0), stop=(xc == XC - 1))
        nc.vector.reciprocal(recip_sb[:, yh:yh + 1], den[:])
        pc = e_psum.tile([P, E], F32, tag=f"e{yh}", name=f"pc{yh}")
        for xc in range(XC):
            nc.tensor.matmul(
                out=pc[:], lhsT=expT_sb[:, xc, yh * P:(yh + 1) * P],
                rhs=enc_sb[:, xc, :], start=(xc == 0), stop=(xc == XC - 1))
        c_sb = out_pool.tile([P, E], F32, tag="c_sb", name=f"c_sb{yh}")
        nc.vector.tensor_scalar_mul(
            out=c_sb[:], in0=pc[:], scalar1=recip_sb[:, yh:yh + 1])
        nc.sync.dma_start(out=c_d[yh * P:(yh + 1) * P, :], in_=c_sb[:])
        for xc in range(XC):
            pt2 = e_psum.tile([P, E], F32, tag=f"e{yh}", name=f"pt2_{yh}_{xc}")
            nc.tensor.transpose(
                out=pt2[:, :P], in_=expT_sb[:, xc, yh * P:(yh + 1) * P],
                identity=ident[:])
            nc.vector.tensor_scalar_mul(
                out=alpha_sb[:, yh, xc * P:(xc + 1) * P], in0=pt2[:, :P],
                scalar1=recip_sb[:, yh:yh + 1])
        nc.sync.dma_start(out=e_d[yh * P:(yh + 1) * P, :],
                          in_=alpha_sb[:, yh, :])

    # (GB, PY, SUB): per (block, e-chunk), PY y's go via the PE piece
    # path (identity matmuls of a step-0-broadcast W plus an
    # inner-broadcast U summed bank-by-bank into double-buffered PSUM
    # pieces that ACT tanh-reads directly); the rest via DVE fp16
    # tensor_scalar, tanh'd in sub-ops of SUB y's (block 0 uses small
    # sub-ops so ACT starts early). Small tail blocks keep the drain
    # chain short.
    blocks = [(32, 6, 8), (32, 6, 26), (32, 6, 26), (32, 6, 26),
              (32, 6, 26), (32, 6, 26), (32, 6, 26), (16, 6, 10),
              (16, 6, 10)]
    assert sum(gb for gb, _, _ in blocks) == Ty
    y0 = 0
    for b, (GB, PY, SUB) in enumerate(blocks):
        slabs = []
        for c in range(EC):
            tslab = tanh_pool.tile([P, GB, Tx], F16, tag="tanh",
                                   name=f"tanh{b}_{c}")
            ndve = GB - PY
            if b == 0 and ndve:
                # Emit the DVE path first: its first small tanh sub-op is
                # ready before the PE piece chain, so ACT starts earlier.
                aslab = add_pool.tile([P, ndve, Tx], F16, tag="add",
                                      name=f"add{b}_{c}")
                for j0 in range(0, ndve, SUB):
                    j1 = min(j0 + SUB, ndve)
                    for j in range(j0, j1):
                        nc.vector.tensor_scalar_add(
                            out=aslab[:, j, :], in0=WsT16_sb[:, c, :],
                            scalar1=UhT_sb[:, c, y0 + PY + j:y0 + PY + j + 1])
                    nc.scalar.activation(out=tslab[:, PY + j0:PY + j1, :],
                                         in_=aslab[:, j0:j1, :], func=TANH)
                ndve = 0
            piece = piece_psum.tile([P, PY * Tx], F32, tag="piece",
                                    name=f"piece{b}_{c}")
            for s in range(PY // 2):
                sub = piece[:, 2 * Tx * s:2 * Tx * (s + 1)]
                nc.tensor.matmul(
                    out=sub,
                    lhsT=ident16[:],
                    rhs=_bcast_add_ap(WsT16_sb[:, c, :], 2, Tx),
                    start=True, stop=False)
                nc.tensor.matmul(
                    out=sub,
                    lhsT=ident16[:],
                    rhs=_bcast_inner_ap(UhT16_sb[:, c, :], y0 + 2 * s,
                                        2, Tx),
                    start=False, stop=True)
            nc.scalar.activation(out=tslab[:, :PY, :], in_=piece[:],
                                 func=TANH)
            if ndve:
                aslab = add_pool.tile([P, ndve, Tx], F16, tag="add",
                                      name=f"add{b}_{c}")
                for j0 in range(0, ndve, SUB):
                    j1 = min(j0 + SUB, ndve)
                    for j in range(j0, j1):
                        nc.vector.tensor_scalar_add(
                            out=aslab[:, j, :], in0=WsT16_sb[:, c, :],
                            scalar1=UhT_sb[:, c, y0 + PY + j:y0 + PY + j + 1])
                    nc.scalar.activation(out=tslab[:, PY + j0:PY + j1, :],
                                         in_=aslab[:, j0:j1, :], func=TANH)
            slabs.append(tslab)
        for j in range(GB):
            y = y0 + j
            for xc in range(XC):
                for c in range(EC):
                    nc.tensor.matmul(
                        out=eT_yh[y // P][:, xc, y % P:y % P + 1],
                        lhsT=slabs[c][:, j, xc * P:(xc + 1) * P],
                        rhs=V16_sb[:, c:c + 1],
                        start=False, stop=False,
                        skip_group_check=True)
        y0 += GB
        if y0 == P:
            _final_half(0)
    _final_half(1)

def _build():
    nc = bacc.Bacc("TRN2", target_bir_lowering=False, debug=False,
                   num_devices=NCORES)
    enc_d = nc.dram_tensor("enc", [Tx, E], F32, kind="ExternalInput").ap()
    dec_d = nc.dram_tensor("dec", [Ty, D], F32, kind="ExternalInput").ap()
    W_d = nc.dram_tensor("W", [E, E], F32, kind="ExternalInput").ap()
    U_d = nc.dram_tensor("U", [D, E], F32, kind="ExternalInput").ap()
    V_d = nc.dram_tensor("V", [E, 1], F32, kind="ExternalInput").ap()
    c_d = nc.dram_tensor("c_out", [Ty, E], F32, kind="ExternalOutput").ap()
    e_d = nc.dram_tensor("e_out", [Ty, Tx], F32, kind="ExternalOutput").ap()

    with tile.TileContext(nc) as tc:
        with ExitStack() as ctx:
            _build_body(tc, ctx, enc_d, dec_d, W_d, U_d, V_d, c_d, e_d)
    nc.compile()
    return nc


def _get_nc():
    global _NC
    if _NC is None:
        _NC = _build()
    return _NC


def kernel(encoder_out_seq, decoder_out_seq, W_a, U_a, V_a):
    enc = np.ascontiguousarray(np.asarray(encoder_out_seq, dtype=np.float32))
    dec = np.ascontiguousarray(np.asarray(decoder_out_seq, dtype=np.float32))
    W = np.ascontiguousarray(np.asarray(W_a, dtype=np.float32))
    U = np.ascontiguousarray(np.asarray(U_a, dtype=np.float32))
    V = np.ascontiguousarray(np.asarray(V_a, dtype=np.float32))

    nc = _get_nc()
    in_maps = [
        {"enc": enc[i], "dec": dec[i], "W": W, "U": U, "V": V}
        for i in range(NCORES)
    ]
    res = run_bass_kernel_spmd(nc, in_maps, list(range(NCORES)))
    global LAST_RESULTS
    LAST_RESULTS = res
    c = np.stack([res.results[i]["c_out"] for i in range(NCORES)])
    e = np.stack([res.results[i]["e_out"] for i in range(NCORES)])
    return c, e


# revision 48
# speedup vs baseline: 1.0079x; 1.0079x over previous
"""Bahdanau additive attention (vectorized) on TRN2 — Bass/Tile kernel.

Problem: nn_AttentionLayer_11055245820581
  e[b,y,x] = softmax_x( sum_e V[e] * tanh(Ws[b,x,e] + Uh[b,y,e]) )
  c[b,y,:] = sum_x e[b,y,x] * enc[b,x,:]
with Ws = enc @ W_a, Uh = dec @ U_a.

Sharding: data-parallel over batch B=8 across the 8 NeuronCores (one
batch element per core). Each core computes its batch's full attention.

Per-core dataflow (the tanh cube Ty*Tx*E = 16.7M elements dominates;
ACT's 1 elem/lane/cycle tanh is the ~110us floor, everything else is
arranged to stay below it; measured ~155us/kernel on HW):
  - broadcast-add WsT[e,x] + UhT[e,y] into fp16 slabs, split per
    (y-block, e-chunk) between DVE (tensor_scalar_add with per-partition
    fp32 scalar, ~283ns per 256-elem op) and the Tensor engine (identity
    matmuls of a step-0-broadcast W plus an inner-broadcast U summed in
    double-buffered PSUM pieces that ACT tanh-reads directly).
  - ACT: one big fp16 Tanh per (y-block, chunk) DVE slab + one per PSUM
    piece; instruction count kept low (352-cycle fixed cost per op).
  - PE: projection with the tanh slab as fp16 stationary [128e, 128x]
    and V fp16 moving (N=1): e'^T lands as [x(partition), y] columns in
    per-y-half PSUM tiles (no PSUM evacuation, M=128 amortizes LDW).
  - softmax per y-half in the transposed layout, overlapped with the
    other half's main loop: ACT Exp -> expT in SBUF; sum over x via
    matmul with a ones vector -> denom[y]; DVE reciprocal; context
    matmul uses unnormalized expT and scales c rows by 1/denom;
    attention weights are PE-transposed back to [y, x] and scaled.
"""

import numpy as np
from contextlib import ExitStack

import concourse.bass as bass
import concourse.bacc as bacc
import concourse.tile as tile
from concourse import mybir
from concourse.bass_utils import run_bass_kernel_spmd

B, Tx, Ty, E, D = 8, 256, 256, 256, 256
P = 128
NCORES = 8
F32 = mybir.dt.float32
F16 = mybir.dt.float16
TANH = mybir.ActivationFunctionType.Tanh
EXP = mybir.ActivationFunctionType.Exp

EC = E // P      # 2 e-chunks
XC = Tx // P     # 2 x-chunks
YC = Ty // P     # 2 y-halves
DC = D // P      # 2 d-chunks

_NC = None
LAST_RESULTS = None


def _bcast_add_ap(t, n_rep, n_inner):
    """AP reading a [P, n_inner] tile as [P, n_rep, n_inner] (repeat dim 1)."""
    return bass.AP(tensor=t.tensor, offset=t.offset,
                   ap=[t.ap[0], [0, n_rep], t.ap[1]])


def _bcast_inner_ap(t, col0, n_rep, n_inner):
    """AP reading tile columns [col0:col0+n_rep] as [P, n_rep, n_inner]
    (each column repeated n_inner times along the innermost dim)."""
    step = t.ap[1][0]
    return bass.AP(tensor=t.tensor, offset=t.offset + col0 * step,
                   ap=[t.ap[0], [step, n_rep], [0, n_inner]])


def _build_body(tc, ctx, enc_d, dec_d, W_d, U_d, V_d, c_d, e_d):
    nc = tc.nc
    from concourse.masks import make_identity

    consts = ctx.enter_context(tc.tile_pool(name="consts", bufs=1))
    add_pool = ctx.enter_context(tc.tile_pool(name="adds", bufs=4))
    tanh_pool = ctx.enter_context(tc.tile_pool(name="tanhs", bufs=4))
    out_pool = ctx.enter_context(tc.tile_pool(name="outs", bufs=2))
    e_psum = ctx.enter_context(tc.tile_pool(name="pe", bufs=1, space="PSUM"))
    piece_psum = ctx.enter_context(tc.tile_pool(name="ppiece", bufs=2, space="PSUM"))
    misc_psum = piece_psum  # setup/final tiles rotate through the piece slots

    # ---- load inputs ----
    enc_sb = consts.tile([P, XC, E], F32)    # [x_in_chunk, (xc), e]
    dec_sb = consts.tile([P, YC, D], F32)
    W_sb = consts.tile([P, EC, E], F32)      # rows e_in
    U_sb = consts.tile([P, DC, E], F32)      # rows d
    V_sb = consts.tile([P, EC], F32)
    for i in range(XC):
        nc.sync.dma_start(out=enc_sb[:, i, :], in_=enc_d[i * P:(i + 1) * P, :])
    for i in range(YC):
        nc.sync.dma_start(out=dec_sb[:, i, :], in_=dec_d[i * P:(i + 1) * P, :])
    for i in range(EC):
        nc.sync.dma_start(out=W_sb[:, i, :], in_=W_d[i * P:(i + 1) * P, :])
    for i in range(DC):
        nc.sync.dma_start(out=U_sb[:, i, :], in_=U_d[i * P:(i + 1) * P, :])
    for i in range(EC):
        nc.sync.dma_start(out=V_sb[:, i:i + 1], in_=V_d[i * P:(i + 1) * P, :])

    ident = consts.tile([P, P], F32)
    make_identity(nc, ident)
    ident16 = consts.tile([P, P], F16)
    nc.vector.tensor_copy(ident16[:], ident[:])
    ones_sb = consts.tile([P, 1], F32)
    nc.vector.memset(ones_sb[:], 1.0)
    V16_sb = consts.tile([P, EC], F16)
    nc.vector.tensor_copy(V16_sb[:], V_sb[:])
    # Trigger the ACT tanh table load during the otherwise-idle prologue.
    warm_sb = consts.tile([P, 1], F32)
    nc.scalar.activation(out=warm_sb[:], in_=ones_sb[:], func=TANH)

    # ---- transpose enc, dec (PE transpose via identity) ----
    encT_sb = consts.tile([P, EC, Tx], F32)  # [e, (ec), x]
    decT_sb = consts.tile([P, DC, Ty], F32)  # [d, (dc), y]
    for src, srcC, dstT, dstC in ((enc_sb, XC, encT_sb, EC),
                                  (dec_sb, YC, decT_sb, DC)):
        for i in range(srcC):          # source partition chunk (x or y)
            for j in range(dstC):      # source free chunk (e or d)
                pt = misc_psum.tile([P, Tx], F32, tag="piece", name="pt")
                nc.tensor.transpose(
                    out=pt[:, :P], in_=src[:, i, j * P:(j + 1) * P],
                    identity=ident[:])
                nc.vector.tensor_copy(dstT[:, j, i * P:(i + 1) * P], pt[:, :P])

    # ---- WsT[e_out, x] = sum_ei W[ei, e_out] * encT[ei, x] ----
    # fp16 WsT/UhT feed the DVE/PE adds; fp32 UhT feeds the DVE
    # per-partition scalar reads (TensorScalar requires fp32 scalars).
    WsT16_sb = consts.tile([P, EC, Tx], F16)
    UhT16_sb = consts.tile([P, EC, Ty], F16)
    UhT_sb = consts.tile([P, EC, Ty], F32)
    for co in range(EC):
        pw = misc_psum.tile([P, Tx], F32, tag="piece", name="pw")
        for ci in range(EC):
            nc.tensor.matmul(
                out=pw[:], lhsT=W_sb[:, ci, co * P:(co + 1) * P],
                rhs=encT_sb[:, ci, :], start=(ci == 0), stop=(ci == EC - 1))
        nc.vector.tensor_copy(WsT16_sb[:, co, :], pw[:])
    for co in range(EC):
        pu = misc_psum.tile([P, Ty], F32, tag="piece", name="pu")
        for ci in range(DC):
            nc.tensor.matmul(
                out=pu[:], lhsT=U_sb[:, ci, co * P:(co + 1) * P],
                rhs=decT_sb[:, ci, :], start=(ci == 0), stop=(ci == DC - 1))
        nc.vector.tensor_copy(UhT_sb[:, co, :], pu[:])
        nc.vector.tensor_copy(UhT16_sb[:, co, :], pu[:])

    # ---- main loop: tanh cube + V projection into e'^T ----
    # e'^T[x, (xc, y)] accumulates into one [128, XC*128] PSUM tile per
    # y-half (1 bank each) so each half's softmax can start while the
    # other half is still being produced.
    eT_yh = [e_psum.tile([P, XC, P], F32, tag=f"e{h}", name=f"eT_yh{h}")
             for h in range(YC)]
    for h in range(YC):
        nc.vector.memset(eT_yh[h][:], 0.0)

    # ---- per-y-half softmax + context + attention-weight output ----
    expT_sb = consts.tile([P, XC, Ty], F32)  # [x, (xc), y]
    recip_sb = consts.tile([P, YC], F32)
    alpha_sb = consts.tile([P, YC, Tx], F32)

    def _final_half(yh):
        for xc in range(XC):
            nc.scalar.activation(out=expT_sb[:, xc, yh * P:(yh + 1) * P],
                                 in_=eT_yh[yh][:, xc, :], func=EXP)
        # Reuse the just-released eT bank of this half for the final
        # tiles (borrowing piece slots here starves ACT of pieces).
        den = e_psum.tile([P, 1], F32, tag=f"e{yh}", name=f"den{yh}")
        for xc in range(XC):
            nc.tensor.matmul(
                out=den[:],
                lhsT=expT_sb[:, xc, yh * P:(yh + 1) * P],
                rhs=ones_sb[:],
                start=(xc == 0), stop=(xc == XC - 1))
        nc.vector.reciprocal(recip_sb[:, yh:yh + 1], den[:])
        pc = e_psum.tile([P, E], F32, tag=f"e{yh}", name=f"pc{yh}")
        for xc in range(XC):
            nc.tensor.matmul(
                out=pc[:], lhsT=expT_sb[:, xc, yh * P:(yh + 1) * P],
                rhs=enc_sb[:, xc, :], start=(xc == 0), stop=(xc == XC - 1))
        c_sb = out_pool.tile([P, E], F32, tag="c_sb", name=f"c_sb{yh}")
        nc.vector.tensor_scalar_mul(
            out=c_sb[:], in0=pc[:], scalar1=recip_sb[:, yh:yh + 1])
        nc.sync.dma_start(out=c_d[yh * P:(yh + 1) * P, :], in_=c_sb[:])
        for xc in range(XC):
            pt2 = e_psum.tile([P, E], F32, tag=f"e{yh}", name=f"pt2_{yh}_{xc}")
            nc.tensor.transpose(
                out=pt2[:, :P], in_=expT_sb[:, xc, yh * P:(yh + 1) * P],
                identity=ident[:])
            nc.vector.tensor_scalar_mul(
                out=alpha_sb[:, yh, xc * P:(xc + 1) * P], in0=pt2[:, :P],
                scalar1=recip_sb[:, yh:yh + 1])
        nc.sync.dma_start(out=e_d[yh * P:(yh + 1) * P, :],
                          in_=alpha_sb[:, yh, :])

    # (GB, PY, SUB): per (block, e-chunk), PY y's go via the PE piece
    # path (identity matmuls of a step-0-broadcast W plus an
    # inner-broadcast U summed bank-by-bank into double-buffered PSUM
    # pieces that ACT tanh-reads directly); the rest via DVE fp16
    # tensor_scalar, tanh'd in sub-ops of SUB y's (block 0 uses small
    # sub-ops so ACT starts early). Small tail blocks keep the drain
    # chain short.
    blocks = [(32, 6, 8), (32, 6, 26), (32, 6, 26), (32, 6, 26),
              (32, 6, 26), (32, 6, 26), (32, 6, 26), (16, 6, 10),
              (16, 6, 10)]
    assert sum(gb for gb, _, _ in blocks) == Ty
    y0 = 0
    for b, (GB, PY, SUB) in enumerate(blocks):
        slabs = []
        for c in range(EC):
            tslab = tanh_pool.tile([P, GB, Tx], F16, tag="tanh",
                                   name=f"tanh{b}_{c}")
            ndve = GB - PY
            if b == 0 and ndve:
                # Emit the DVE path first: its first small tanh sub-op is
                # ready before the PE piece chain, so ACT starts earlier.
                aslab = add_pool.tile([P, ndve, Tx], F16, tag="add",
                                      name=f"add{b}_{c}")
                for j0 in range(0, ndve, SUB):
                    j1 = min(j0 + SUB, ndve)
                    for j in range(j0, j1):
                        nc.vector.tensor_scalar_add(
                            out=aslab[:, j, :], in0=WsT16_sb[:, c, :],
                            scalar1=UhT_sb[:, c, y0 + PY + j:y0 + PY + j + 1])
                    nc.scalar.activation(out=tslab[:, PY + j0:PY + j1, :],
                                         in_=aslab[:, j0:j1, :], func=TANH)
                ndve = 0
            piece = piece_psum.tile([P, PY * Tx], F32, tag="piece",
                                    name=f"piece{b}_{c}")
            for s in range(PY // 2):
                sub = piece[:, 2 * Tx * s:2 * Tx * (s + 1)]
                nc.tensor.matmul(
                    out=sub,
                    lhsT=ident16[:],
                    rhs=_bcast_add_ap(WsT16_sb[:, c, :], 2, Tx),
                    start=True, stop=False)
                nc.tensor.matmul(
                    out=sub,
                    lhsT=ident16[:],
                    rhs=_bcast_inner_ap(UhT16_sb[:, c, :], y0 + 2 * s,
                                        2, Tx),
                    start=False, stop=True)
            nc.scalar.activation(out=tslab[:, :PY, :], in_=piece[:],
                                 func=TANH)
            if ndve:
                aslab = add_pool.tile([P, ndve, Tx], F16, tag="add",
                                      name=f"add{b}_{c}")
                for j0 in range(0, ndve, SUB):
                    j1 = min(j0 + SUB, ndve)
                    for j in range(j0, j1):
                        nc.vector.tensor_scalar_add(
                            out=aslab[:, j, :], in0=WsT16_sb[:, c, :],
                            scalar1=UhT_sb[:, c, y0 + PY + j:y0 + PY + j + 1])
                    nc.scalar.activation(out=tslab[:, PY + j0:PY + j1, :],
                                         in_=aslab[:, j0:j1, :], func=TANH)
            slabs.append(tslab)
        for j in range(GB):
            y = y0 + j
            for xc in range(XC):
                for c in range(EC):
                    nc.tensor.matmul(
                        out=eT_yh[y // P][:, xc, y % P:y % P + 1],
                        lhsT=slabs[c][:, j, xc * P:(xc + 1) * P],
                        rhs=V16_sb[:, c:c + 1],
                        start=False, stop=False,
                        skip_group_check=True)
        y0 += GB
        if y0 == P:
            _final_half(0)
    _final_half(1)

def _build():
    nc = bacc.Bacc("TRN2", target_bir_lowering=False, debug=False,
                   num_devices=NCORES)
    enc_d = nc.dram_tensor("enc", [Tx, E], F32, kind="ExternalInput").ap()
    dec_d = nc.dram_tensor("dec", [Ty, D], F32, kind="ExternalInput").ap()
    W_d = nc.dram_tensor("W", [E, E], F32, kind="ExternalInput").ap()
    U_d = nc.dram_tensor("U", [D, E], F32, kind="ExternalInput").ap()
    V_d = nc.dram_tensor("V", [E, 1], F32, kind="ExternalInput").ap()
    c_d = nc.dram_tensor("c_out", [Ty, E], F32, kind="ExternalOutput").ap()
    e_d = nc.dram_tensor("e_out", [Ty, Tx], F32, kind="ExternalOutput").ap()

    with tile.TileContext(nc) as tc:
        with ExitStack() as ctx:
            _build_body(tc, ctx, enc_d, dec_d, W_d, U_d, V_d, c_d, e_d)
    nc.compile()
    return nc


def _get_nc():
    global _NC
    if _NC is None:
        _NC = _build()
    return _NC


def kernel(encoder_out_seq, decoder_out_seq, W_a, U_a, V_a):
    enc = np.ascontiguousarray(np.asarray(encoder_out_seq, dtype=np.float32))
    dec = np.ascontiguousarray(np.asarray(decoder_out_seq, dtype=np.float32))
    W = np.ascontiguousarray(np.asarray(W_a, dtype=np.float32))
    U = np.ascontiguousarray(np.asarray(U_a, dtype=np.float32))
    V = np.ascontiguousarray(np.asarray(V_a, dtype=np.float32))

    nc = _get_nc()
    in_maps = [
        {"enc": enc[i], "dec": dec[i], "W": W, "U": U, "V": V}
        for i in range(NCORES)
    ]
    res = run_bass_kernel_spmd(nc, in_maps, list(range(NCORES)))
    global LAST_RESULTS
    LAST_RESULTS = res
    c = np.stack([res.results[i]["c_out"] for i in range(NCORES)])
    e = np.stack([res.results[i]["e_out"] for i in range(NCORES)])
    return c, e
